# revision 1
# baseline (speedup 1.0000x reference)
"""CMFM loss kernel for Trainium2 (8 NeuronCores, Bass/Tile).

Math: for inputs f_v, f_a [B,T,D] with vn/an the D-normalized tensors,
  cos[b,t]    = s_va / (sqrt(s_vv)*sqrt(s_aa))          (per-timestep term)
  cross[i,j]  = (1/T) sum_t vn[i,t,:].an[j,t,:]
  sum_{i!=j} cross = (1/T)*(sum_t V_t.A_t  -  sum_{b,t} cos[b,t])
where V_t = sum_b vn[b,t,:], A_t = sum_b an[b,t,:].  So the BxB cross term
only needs the batch-summed normalized features -> data-parallel over B with
a tiny cross-core combine of the per-core partial V/A sums and cos stats.

Per core (8 batch rows): 64 tiles of [128 t-partitions, 256 d-free],
loaded as one 1MB DMA per (row, tensor) to amortize the ~650ns per-DMA
issue cost.  Engine split (measured on HW, ns per [128,256] op):
  ACT: Square+accum (632), Sqrt            - same table set, no reloads
  DVE: scalar_tensor_tensor fused ops (352): s_va mul+reduce, V/A
       scale-accumulate; reciprocal
  sq_a runs on ACT for 6/8 tiles, DVE for 2/8 (engine balance).
Emission is software-pipelined in 4-tile groups (stage-2 accumulate lags
one group) so the in-order DVE stream never blocks on ACT.
Outputs per core: cos stats [128,64], V_acc/A_acc [128,8*256] partials.
Host: sums the 8 partial V/A tensors, dots them, applies label masks.

Measured per-core pass: ~91 us vs ~55 us DMA roofline (16.8MB/core),
ACT/DVE busy ~72-74 us each -- compute-balanced slightly above the
memory roofline.

Runtime quirks discovered on this stack: InstTensorTensorReduce crashes
the NRT; Pool rejects TensorScalarPtr; Pool TT-mult offload measures far
worse than the cost model predicts (Q7 chains don't pipeline); ACT
Rsqrt/Reciprocal are banned in bass (accuracy) -> Sqrt + DVE reciprocal.
"""

import numpy as np

import concourse.bacc as bacc
import concourse.bass as bass
import concourse.tile as tile
from concourse import mybir
from concourse.bass_utils import run_bass_kernel_spmd

ALPHA, BETA, GAMMA = 2.0, 2.0, 1.0
B, T, D = 64, 1024, 256
N_CORES = 8
B_LOC = B // N_CORES          # 8 batch rows per core
P = 128                       # SBUF partitions
TCH = T // P                  # 8 t-chunks per batch row
NTILES = B_LOC * TCH          # 64 tiles per core

F32 = mybir.dt.float32
MULT = mybir.AluOpType.mult
ADD = mybir.AluOpType.add

# fraction of tiles whose sq_a runs on ACT (rest on DVE) - engine balance knob
import os as _os
SQA_ACT_NUM, SQA_ACT_DEN = 6, 8
if _os.environ.get("K_SQA"):
    SQA_ACT_NUM, SQA_ACT_DEN = (int(x) for x in _os.environ["K_SQA"].split("/"))
# Pool offload of the v*a multiply (reduced on DVE per group)
POOL_VA = bool(int(_os.environ.get("K_POOL_VA", "0")))

_CACHE = {}

# Results of the most recent device run (for test harness introspection).
LAST_RESULTS = None


def _build_nc(repeat=1, loop_n=1):
    """Build the per-core Bass program.

    repeat>1 (static unroll) or loop_n>1 (device-side For_i loop) re-run
    the whole compute pass on the same data for wall-clock timing:
    (wall[N] - wall[M]) / (N - M) isolates device time per pass from
    dispatch/transfer overhead.  Stats are simply overwritten; V/A
    accumulate extra times but timing variants are never used for values.
    """
    nc = bacc.Bacc("TRN2", debug=False)

    v = nc.dram_tensor("v", [B_LOC, T, D], F32, kind="ExternalInput").ap()
    a = nc.dram_tensor("a", [B_LOC, T, D], F32, kind="ExternalInput").ap()
    cos_out = nc.dram_tensor("cos_stat", [P, NTILES], F32, kind="ExternalOutput").ap()
    vacc_out = nc.dram_tensor("v_acc", [P, TCH * D], F32, kind="ExternalOutput").ap()
    aacc_out = nc.dram_tensor("a_acc", [P, TCH * D], F32, kind="ExternalOutput").ap()

    with tile.TileContext(nc) as tc:
        with (
            tc.tile_pool(name="io", bufs=int(__import__("os").environ.get("K_IO_BUFS", "4"))) as io_pool,
            tc.tile_pool(name="scratch", bufs=int(__import__("os").environ.get("K_SCRATCH_BUFS", "6"))) as scratch,
            tc.tile_pool(name="small", bufs=int(__import__("os").environ.get("K_SMALL_BUFS", "12"))) as small,
            tc.tile_pool(name="acc", bufs=1) as accp,
        ):
            v_accT = accp.tile([P, TCH, D], F32)   # V partial, t=tc*128+p
            a_accT = accp.tile([P, TCH, D], F32)
            sva_stat = accp.tile([P, NTILES], F32)     # col = b*TCH+tc
            inv_stat = accp.tile([P, NTILES, 2], F32)  # (1/|v|, 1/|a|)
            cos_stat = accp.tile([P, NTILES], F32)

            nc.gpsimd.memset(v_accT[:], 0.0)
            nc.gpsimd.memset(a_accT[:], 0.0)

            import contextlib
            loop_ctx = (
                tc.For_i(
                    0, loop_n, 1,
                    hint_engines=(
                        mybir.EngineType.DVE,
                        mybir.EngineType.Activation,
                        mybir.EngineType.SP,
                    ),
                )
                if loop_n > 1
                else contextlib.nullcontext()
            )
            GRP = 4  # tiles per pipeline group (half a row)
            with loop_ctx:
              for _ in range(repeat):
                # software pipeline over groups of GRP tiles: stage 1
                # (loads, squares, s_va, norms) for group g is emitted
                # before stage 2 (V/A scale-accumulate) of group g-1, so
                # the in-order DVE stream never head-of-line blocks on
                # ACT finishing a group's squares.
                groups = [(b, tcs) for b in range(B_LOC)
                          for tcs in range(0, TCH, GRP)]
                pend = None   # (vt_s, at_s, b, tcs) awaiting stage 2
                supers = None
                for g in range(len(groups) + 1):
                    if g < len(groups):
                        b, tcs = groups[g]
                        if tcs == 0:
                            # one 1MB DMA per (b, tensor): [128, 8, 256]
                            # supertile, (p, j, d) = x[b, j*128+p, d].
                            # Row 0 is split in half so compute starts
                            # ~3us sooner (ramp cut).
                            vt_s = io_pool.tile([P, TCH, D], F32, tag="vt")
                            at_s = io_pool.tile([P, TCH, D], F32, tag="at")
                            vr = v[b].rearrange("(j p) d -> p j d", p=P)
                            ar = a[b].rearrange("(j p) d -> p j d", p=P)
                            nspl = 2 if b == 0 else 1
                            hh = TCH // nspl
                            for s_ in range(nspl):
                                nc.sync.dma_start(
                                    out=vt_s[:, s_ * hh:(s_ + 1) * hh, :],
                                    in_=vr[:, s_ * hh:(s_ + 1) * hh, :])
                                nc.sync.dma_start(
                                    out=at_s[:, s_ * hh:(s_ + 1) * hh, :],
                                    in_=ar[:, s_ * hh:(s_ + 1) * hh, :])
                            supers = (vt_s, at_s)
                        vt_s, at_s = supers

                        # norm^2 pairs for this group: [128, GRP, 2]
                        pair = small.tile([P, GRP, 2], F32, tag="pair")
                        prodg = None
                        if POOL_VA:
                            prodg = scratch.tile([P, GRP, D], F32, tag="prodg")
                        for k in range(GRP):
                            tci = tcs + k
                            idx = b * TCH + tci
                            vt = vt_s[:, tci, :]
                            at = at_s[:, tci, :]

                            # s_vv on ACT (Square in sqrt_and_others set)
                            sqv = scratch.tile([P, D], F32, tag="sqv")
                            nc.scalar.activation(
                                out=sqv[:], in_=vt,
                                func=mybir.ActivationFunctionType.Square,
                                accum_out=pair[:, k, 0:1],
                            )
                            # s_aa: ACT or DVE depending on balance knob.
                            # DVE-handled tiles go FIRST in the group so
                            # the group sqrt isn't gated on DVE's tail.
                            sqa = scratch.tile([P, D], F32, tag="sqa")
                            if idx % SQA_ACT_DEN >= SQA_ACT_DEN - SQA_ACT_NUM:
                                nc.scalar.activation(
                                    out=sqa[:], in_=at,
                                    func=mybir.ActivationFunctionType.Square,
                                    accum_out=pair[:, k, 1:2],
                                )
                            else:
                                nc.vector.scalar_tensor_tensor(
                                    out=sqa[:], in0=at, scalar=1.0, in1=at,
                                    op0=MULT, op1=MULT,
                                    accum_out=pair[:, k, 1:2],
                                )

                            # s_va: either fused (v*1)*a with accum on
                            # DVE, or (Pool offload) a Pool TT multiply
                            # into a group supertile reduced below on DVE.
                            if POOL_VA:
                                nc.gpsimd.tensor_mul(
                                    out=prodg[:, k, :], in0=vt, in1=at)
                            else:
                                prod = scratch.tile([P, D], F32, tag="prod")
                                nc.vector.scalar_tensor_tensor(
                                    out=prod[:], in0=vt, scalar=1.0, in1=at,
                                    op0=MULT, op1=MULT,
                                    accum_out=sva_stat[:, idx:idx + 1],
                                )

                    if pend is not None:
                        pvt_s, pat_s, pb, ptcs = pend
                        for k in range(GRP):
                            tci = ptcs + k
                            idx = pb * TCH + tci
                            # V_acc[:,tci,:] += v * (1/|v|)  (fused DVE)
                            nc.vector.scalar_tensor_tensor(
                                out=v_accT[:, tci, :], in0=pvt_s[:, tci, :],
                                scalar=inv_stat[:, idx, 0:1],
                                in1=v_accT[:, tci, :],
                                op0=MULT, op1=ADD,
                            )
                            nc.vector.scalar_tensor_tensor(
                                out=a_accT[:, tci, :], in0=pat_s[:, tci, :],
                                scalar=inv_stat[:, idx, 1:2],
                                in1=a_accT[:, tci, :],
                                op0=MULT, op1=ADD,
                            )
                        if pb == B_LOC - 1 and repeat == 1 and loop_n == 1:
                            # b=7 is the final update for these tc slices:
                            # stream this half out now to shorten the tail
                            lo, hi = ptcs * D, (ptcs + GRP) * D
                            nc.sync.dma_start(
                                out=vacc_out[:, lo:hi],
                                in_=v_accT[:, ptcs:ptcs + GRP, :])
                            nc.sync.dma_start(
                                out=aacc_out[:, lo:hi],
                                in_=a_accT[:, ptcs:ptcs + GRP, :])

                    if g < len(groups):
                        if POOL_VA:
                            i0 = b * TCH + tcs
                            nc.vector.tensor_reduce(
                                out=sva_stat[:, i0:i0 + GRP], in_=prodg[:],
                                axis=mybir.AxisListType.X,
                                op=mybir.AluOpType.add)
                        # batched norm + reciprocal for this group,
                        # emitted AFTER the previous group's affines so
                        # the in-order DVE stream doesn't head-of-line
                        # block on ACT finishing this group's squares.
                        norm = small.tile([P, GRP, 2], F32, tag="norm")
                        nc.scalar.activation(
                            out=norm[:], in_=pair[:],
                            func=mybir.ActivationFunctionType.Sqrt,
                        )
                        i0 = b * TCH + tcs
                        nc.vector.reciprocal(
                            out=inv_stat[:, i0:i0 + GRP, :], in_=norm[:])
                        pend = (vt_s, at_s, b, tcs)
                    else:
                        pend = None

            # cos = s_va * inv_v * inv_a  (deferred, two [128,64] DVE ops)
            ii = accp.tile([P, NTILES], F32)
            nc.vector.tensor_mul(
                out=ii[:], in0=inv_stat[:, :, 0], in1=inv_stat[:, :, 1]
            )
            nc.vector.tensor_mul(out=cos_stat[:], in0=ii[:], in1=sva_stat[:])

            nc.sync.dma_start(out=cos_out[:, :], in_=cos_stat[:])
            if repeat != 1 or loop_n != 1:
                nc.sync.dma_start(out=vacc_out[:, :], in_=v_accT[:])
                nc.sync.dma_start(out=aacc_out[:, :], in_=a_accT[:])

    nc.compile()
    return nc


def _get_nc(repeat=1, loop_n=1):
    key = ("nc", repeat, loop_n)
    if key not in _CACHE:
        _CACHE[key] = _build_nc(repeat, loop_n)
    return _CACHE[key]


def _run(nc, f_v, f_a):
    in_maps = [
        {
            "v": np.ascontiguousarray(f_v[c * B_LOC:(c + 1) * B_LOC]),
            "a": np.ascontiguousarray(f_a[c * B_LOC:(c + 1) * B_LOC]),
        }
        for c in range(N_CORES)
    ]
    return run_bass_kernel_spmd(nc, in_maps, core_ids=list(range(N_CORES)))


def kernel(f_v, f_a, labels):
    global LAST_RESULTS
    f_v = np.asarray(f_v, dtype=np.float32)
    f_a = np.asarray(f_a, dtype=np.float32)
    labels = np.asarray(labels)

    res = _run(_get_nc(), f_v, f_a)
    LAST_RESULTS = res
    out = res.results

    # cos_stat[c][p, b_loc*TCH+tc] = cos(b=c*B_LOC+b_loc, t=tc*128+p)
    cos = np.stack([out[c]["cos_stat"] for c in range(N_CORES)])
    cos = cos.reshape(N_CORES, P, B_LOC, TCH)
    row_cos = cos.sum(axis=(1, 3), dtype=np.float64).reshape(B)

    v_acc = np.zeros((P, TCH * D), np.float64)
    a_acc = np.zeros((P, TCH * D), np.float64)
    for c in range(N_CORES):
        v_acc += out[c]["v_acc"]
        a_acc += out[c]["a_acc"]
    cross_sum = float((v_acc * a_acc).sum())   # = sum_t V_t . A_t

    pos = labels == 0
    n_pos = int(pos.sum())
    n_neg = B - n_pos

    loss_pos = ALPHA * (n_pos * T - row_cos[pos].sum())
    loss_neg = BETA * row_cos[~pos].sum()
    loss_neg += GAMMA * (cross_sum - row_cos.sum()) / T
    cnt_pos = n_pos * T
    cnt_neg = n_neg * T + B * (B - 1)

    loss = 0.0
    if cnt_pos > 0:
        loss += loss_pos / max(cnt_pos, 1.0)
    if cnt_neg > 0:
        loss += loss_neg / max(cnt_neg, 1.0)
    return np.float32(loss)

